# revision 26
# baseline (speedup 1.0000x reference)
"""Causal multi-head attention (B=2, H=16, S=2048, D=64, fp32 I/O) on 8 TRN2
NeuronCores.

Sharding: batch*heads (32 units) split 4-per-core - embarrassingly parallel,
no collectives.

v2 of the per-core kernel (bf16 compute, fp32 PSUM accumulation); measured
~84us typical (vs 109-111us for v1; watch out: run-to-run environment/DVFS
throttling noise of up to +18us was observed on identical binaries - the
`throttle_*` / `ham` fields of the ntff json tell the two apart).
Structural changes vs v1:
  - HOST pre-TRANSPOSES the K|Q layout entirely: the DRAM tensor kqt holds,
    per (head, q-half), exactly the SBUF image the compute wants
    ([128 part, 4 K-pair-slabs x 128 | 8 Q-dup-tiles x 128]), so inputs
    arrive via PLAIN contiguous DMAs (3KB/partition lines) instead of the
    v1 chain of serialized Sync-ring DMA_TRANSPOSEs (which cost a 7.5us PE
    stall at t=15-22us plus a HAM half-duty window to 28.7us).
  - ALL input loads ride ONE hardware DMA queue (the sync ring) in exact
    consumption order (K0 | Q0-tiles-0:4 | Q0-tiles-4:8 | V0 | kq-ph1 |
    head1... ), so the first-needed bytes never lose HBM bandwidth to
    prefetch round-robin across queues. Output stores ride the same ring;
    the scalar queue stays dedicated to evictions.
  - HOST pre-PADS V to 128 stationary columns (V | ones | zeros): V loads
    become fully contiguous 4KB-line DMAs, the on-chip ones-column memset
    disappears, and out^T junk rows become zeros.
  - The per-head epilogue (out^T transpose + reciprocal + scale) moved to
    the HOST: the kernel stores out^T rows 0:65 (64 out dims + denominator
    row from the ones column) straight from the bfo buffer, one plain DMA
    per q-half. Removes the onat DMA_TRANSPOSE (sync ring), reciprocal and
    scale (DVE) per head, and cuts the tail from ~8.4us to ~3us.
  - CROSS-HALF SOFTWARE PIPELINING: each half's tail (its last PV_LAG PV
    batches + 2 bfo evictions) is returned as closures and interleaved
    into the NEXT half's first pairs, so the PE's half-start window (3
    fresh QK pairs racing 6us of evictions) is filled with carried PV
    work. This removed the ~1.2us PE stall per half boundary AND the HAM
    half-duty windows those stalls triggered (full 8/8 duty for the whole
    compute span now).
  - PV lags THREE pairs (not two): PV sits in the in-order PE queue, so a
    deeper lag keeps it from head-of-line blocking on eviction completion.
  - PE warm-up (HAM duty-cycle hold-open) decoupled from the tri01 mask:
    warm weights come from a gpsimd-memset tile so warm matmuls start at
    ~6.5us and run until the first real QK (~10.2us).
  - Last head processes halves (1,0) and the final half's block-0
    eviction + store overlap the block-1 PV tail.
Kept from v1: transposed scores with dual-row-group QK pairing, ScalarE-
exact-Exp / VectorE-Schraudolph eviction balancing, diag-block masks,
PSUM slot rotation (3 score slots + 1 out slot).
Measured dead ends this session: GpSimd cannot access PSUM (BIR verifier
rejects); matmul output must be fp32 (no bf16-PSUM -> no DVE 2x_1p evict);
merging PV matmuls across eviction chunks (per-pair ptiles) coarsens the
evict->PV dependencies enough to re-trigger HAM dips (92.4us); PV_LAG=4
measured worse (86.9); splitting V0 into (0:2)+(2:8) pieces measured
worse (86.2, possibly noise). Split-k PV is blocked by the 8-PSUM-bank
wall (2 accumulators + 3 fp32 score slots = 10 banks) and by HAM
punishing an underutilized PE with half-duty.
"""

import numpy as np

import concourse.bass as bass
import concourse.mybir as mybir
import concourse.tile as tile
from concourse import bacc
from concourse.bass_utils import run_bass_kernel_spmd
from concourse.masks import make_upper_triangular
from concourse.alu_op_type import AluOpType

B, H, S, D = 2, 16, 2048, 64
N_CORES = 8
HPC = (B * H) // N_CORES  # heads per core
NT = S // 128  # 16 k/q blocks of 128
FP32 = mybir.dt.float32
BF16 = mybir.dt.bfloat16
I16 = mybir.dt.int16

LOG2E = 1.4426950408889634
EXP_A = 128.0 * LOG2E / 8.0  # folds softmax scale 1/sqrt(64) into the affine
EXP_B = 127.0 * 128.0 - 5.5
MASK_NEG = -30720.0

SC_BUFS = 3
OPS_BUFS = 1
WARM_MMS = 16
PV_LAG = 3


def _act_cost(el):
    return (el + 170) * 0.8333


def _dve_cost(el, fast=False):
    # fast: DVE 2x_1p mode (all non-scalar operands 2-byte + packed) -
    # applies to the non-diag Schraudolph path on bf16 PSUM scores
    if fast:
        return (0.5 * el + 140) * 1.0417
    return (el + 140) * 1.0417


def build_attention():
    nc = bacc.Bacc("TRN2", target_bir_lowering=False)
    # host supplies bf16 already in the SBUF-image layout per (head, q-half):
    # [128 part, (4 K-pair-slab banks x 128) | (8 Q-dup-tile banks x 128)]
    # so K^T pair-slabs + Q^T (dup'd on both partition halves) land via plain
    # contiguous DMAs - no on-chip transposes, no casts.
    kqt_d = nc.dram_tensor("kqt", [HPC, 2, 128, 1536], BF16, kind="ExternalInput")
    # V padded to 128 cols on host: 0:64 = V^blk, 64 = ones, 65:128 = zeros
    v_d = nc.dram_tensor("value", [HPC, 128, NT * 128], BF16, kind="ExternalInput")
    # out^T per (head, q-half): rows 0:64 = out^T (unscaled), row 64 = softmax
    # denominator; host does the reciprocal-scale and final transpose.
    o_d = nc.dram_tensor("out", [HPC, 2, 65, 1024], BF16, kind="ExternalOutput")

    T = {"act": 0.0, "dve": 0.0, "gps": 0.0}

    with tile.TileContext(nc) as tc:
        with (
            tc.tile_pool(name="singles", bufs=1) as singles,
            tc.tile_pool(name="slab", bufs=3) as slab_pool,
            tc.tile_pool(name="sp1", bufs=1) as sp1_pool,
            tc.tile_pool(name="vp", bufs=3) as v_pool,
            tc.tile_pool(name="sp2", bufs=1) as sp2_pool,
            tc.tile_pool(name="pt", bufs=12) as pt_pool,
            tc.tile_pool(name="ep", bufs=4) as ep_pool,
            tc.tile_pool(name="sc", bufs=SC_BUFS, space="PSUM") as sc_pool,
            tc.tile_pool(name="ops", bufs=OPS_BUFS, space="PSUM") as ops_pool,
        ):
            # spacers: keep DMA-written pools non-adjacent so conservative
            # range-overlap dep tracking never chains unrelated DMAs
            sp1_pool.tile([128, 64], BF16, tag="sp", name="sp1t")
            sp2_pool.tile([128, 64], BF16, tag="sp", name="sp2t")

            # ---- per-head tiles ----
            def alloc_head():
                return {
                    # transposed: [:, 0:4, :] = kslab pairs, [:, 4:12, :] = qt
                    "trs0": slab_pool.tile([128, 12, 128], BF16, tag="trs0", name="t0"),
                    "trs1": slab_pool.tile([128, 12, 128], BF16, tag="trs1", name="t1"),
                    # V padded to 128 cols: 0:64 = V, 64 = ones, 65:128 zeros
                    "vaug": v_pool.tile([128, NT, 128], BF16, tag="vaug", name="va"),
                }

            # ALL input loads go on the sync ring (one hardware DMA queue)
            # in exact consumption order: queue order = transfer order, so
            # the first-needed bytes never lose bandwidth to prefetch.
            def emit_load_kq(h, hd, ph, split=False):
                trs = hd[f"trs{ph}"]
                src = kqt_d[h, ph]
                if split:
                    # K part first, then Q tiles 0-3, so the first QK pair
                    # can fire asap
                    nc.sync.dma_start(out=trs[:, 0:4, :], in_=src[:, 0:512])
                    nc.sync.dma_start(out=trs[:, 4:8, :], in_=src[:, 512:1024])
                    nc.sync.dma_start(out=trs[:, 8:12, :], in_=src[:, 1024:1536])
                else:
                    nc.sync.dma_start(out=trs, in_=src)

            def emit_load_v(h, hd, blocks=None):
                if blocks is None:
                    nc.sync.dma_start(out=hd["vaug"], in_=v_d[h])
                else:
                    a, b = blocks
                    nc.sync.dma_start(
                        out=hd["vaug"][:, a:b, :],
                        in_=v_d[h][:, 128 * a : 128 * b],
                    )

            # ---- head-0 loads FIRST so the DMA queue issues before any
            # mask/warm work ----
            hd = [None] * HPC
            hd[0] = alloc_head()
            emit_load_kq(0, hd[0], 0, split=True)
            emit_load_v(0, hd[0], blocks=(0, 2))
            emit_load_v(0, hd[0], blocks=(2, 8))
            emit_load_kq(0, hd[0], 1)
            emit_load_v(0, hd[0], blocks=(8, NT))

            # PE HAM warm-up: vector-memset weights (no gpsimd dependency)
            # keep the clock gate open until the first real QK work
            warm = singles.tile([128, 256], BF16, tag="warm")
            nc.gpsimd.memset(warm, 0.0)
            wslot = sc_pool.tile([128, 2, 512], FP32, tag="slot", name="wslot")
            for _ in range(WARM_MMS):
                nc.tensor.matmul(
                    wslot[:, 0, 0:256], warm[:, 0:128], warm, start=True, stop=True
                )

            # ---- one-time masks ----
            tri01 = singles.tile([128, 128], BF16, tag="tri01")
            make_upper_triangular(nc, tri01, val=1.0, diag=True)
            # VectorE fast-exp bias-with-mask: B where keep, -30k where masked
            mbt = singles.tile([128, 128], FP32, tag="mbt")
            nc.gpsimd.memset(mbt, MASK_NEG)
            nc.gpsimd.affine_select(
                out=mbt,
                in_=mbt,
                compare_op=mybir.AluOpType.is_gt,
                fill=EXP_B,
                base=0,
                pattern=[[-1, 128]],
                channel_multiplier=1,
            )
            mb = singles.tile([128, 2, 512], FP32, tag="mb")
            nc.vector.memset(mb, EXP_B)
            nc.vector.tensor_copy(mb[:, 0, 0:128], mbt)
            nc.vector.tensor_copy(mb[:, 1, 128:256], mbt)

            def kslab_ap(hd, rows, kj):
                j = kj // 2
                return hd[f"trs{j // 4}"][rows : rows + 64, j % 4, :]

            def qt_ap(hd, rows, ca, cb):
                hfq = ca // 1024
                trs_f = hd[f"trs{hfq}"].rearrange("p b c -> p (b c)")
                return trs_f[
                    rows : rows + 64,
                    512 + ca - 1024 * hfq : 512 + cb - 1024 * hfq,
                ]

            # ---- eviction units (static greedy ACT/DVE balance) ----
            def evict_unit(slot, ptile, off, cols, diag):
                el = 2 * cols
                if T["act"] + _act_cost(el) <= T["dve"] + _dve_cost(el):
                    T["act"] += _act_cost(el)
                    nc.scalar.activation(
                        ptile[:, :, off : off + cols],
                        slot[:, :, 0:cols],
                        mybir.ActivationFunctionType.Exp,
                        scale=0.125,
                    )
                    if diag:
                        for lane in range(2):
                            nc.gpsimd.tensor_mul(
                                ptile[:, lane, 128 * lane : 128 * lane + 128],
                                ptile[:, lane, 128 * lane : 128 * lane + 128],
                                tri01,
                            )
                        T["gps"] += 940
                else:
                    T["dve"] += _dve_cost(el)
                    p16 = ptile.bitcast(I16)
                    if diag:
                        nc.vector.scalar_tensor_tensor(
                            out=p16[:, :, off : off + cols],
                            in0=slot[:, :, 0:cols],
                            scalar=EXP_A,
                            in1=mb[:, :, 0:cols],
                            op0=AluOpType.mult,
                            op1=AluOpType.add,
                        )
                    else:
                        nc.vector.tensor_scalar(
                            out=p16[:, :, off : off + cols],
                            in0=slot[:, :, 0:cols],
                            scalar1=EXP_A,
                            scalar2=EXP_B,
                            op0=AluOpType.mult,
                            op1=AluOpType.add,
                        )

            def bfo_unit(dst, src):
                # NOTE: GpSimd cannot access PSUM (BIR verifier) - bfo must
                # stay on ACT/DVE
                el = 512
                if T["act"] + _act_cost(el) + 400 <= T["dve"] + _dve_cost(el):
                    T["act"] += _act_cost(el)
                    nc.scalar.copy(dst, src)
                else:
                    T["dve"] += _dve_cost(el)
                    nc.vector.tensor_copy(dst, src)

            # ---- one (head, half) of compute ----
            # Returns the half's TAIL as a list of closures: the last two
            # pairs' PV batches + the two bfo (out^T) evictions. The caller
            # feeds them back as `carry` into the NEXT half, which interleaves
            # them into its first pairs (pj 0-1, before its own PV exists) -
            # this fills the PE's half-start window and lets the eviction
            # engines drain the new half's first slots without stalling QK.
            def emit_half(h, hd, hf, bfo, carry):
                q0 = 1024 * hf
                q1 = q0 + 1024
                kj_hi = 8 * (hf + 1)
                last_kj = [
                    max(
                        kj
                        for kj in range(kj_hi)
                        if max(q0, 128 * kj) < q0 + 512 * (b + 1)
                    )
                    for b in range(2)
                ]

                outps = ops_pool.tile([128, 2, 512], FP32, tag="outps")
                outps_f = outps.rearrange("p a b -> p (a b)")

                def emit_pv(pair, qas, chunks):
                    for lane, (kj, qa) in enumerate(zip(pair, qas)):
                        for ca, cb, ptile in chunks:
                            lo = max(ca, qa)
                            while lo < cb:
                                hi = min(cb, q0 + 512 * ((lo - q0) // 512 + 1))
                                bk = (lo - q0) // 512
                                nc.tensor.matmul(
                                    outps_f[:, lo - q0 : hi - q0],
                                    hd["vaug"][:, kj, :],
                                    ptile[:, lane, lo - ca : hi - ca],
                                    start=(kj == 0),
                                    stop=(kj == last_kj[bk]),
                                )
                                lo = hi

                pending = []
                for pj in range(kj_hi // 2):
                    pair = (2 * pj, 2 * pj + 1)
                    qas = [max(q0, 128 * kj) for kj in pair]
                    diag0 = 128 * pair[0] >= q0
                    chunks = []
                    for ca in range(qas[0], q1, 512):
                        cb = min(ca + 512, q1)
                        cols = cb - ca
                        slot = sc_pool.tile(
                            [128, 2, 512], FP32, tag="slot", name="slot"
                        )
                        for lane, (kj, qa) in enumerate(zip(pair, qas)):
                            lo = max(ca, qa)
                            if lo >= cb:
                                continue
                            rows = (kj % 2) * 64
                            nc.tensor.matmul(
                                slot[:, lane, lo - ca : cols],
                                kslab_ap(hd, rows, kj),
                                qt_ap(hd, rows, lo, cb),
                                start=True,
                                stop=True,
                            )
                        ptile = pt_pool.tile(
                            [128, 2, 512], BF16, tag="ptile", name="ptile"
                        )
                        evict_unit(slot, ptile, 0, cols, diag0 and ca == qas[0])
                        chunks.append((ca, cb, ptile))
                        # one carried tail unit from the previous half per
                        # fresh chunk (the chunk slots in pj 0-2 cover the
                        # carry units)
                        if pj < 3 and carry:
                            carry.pop(0)()
                    pending.append((pair, qas, chunks))
                    # PV lags PV_LAG pairs so evict(j) finishes while
                    # subsequent QK pairs stream (avoids in-order PE queue
                    # head-of-line blocking on the eviction engines)
                    if pj >= PV_LAG:
                        emit_pv(*pending.pop(0))
                while carry:
                    carry.pop(0)()

                tail = [
                    (lambda a=args: emit_pv(*a)) for args in pending
                ]
                tail.append(
                    lambda: bfo_unit(bfo[:, hf, 0, :], outps_f[0:80, 0:512])
                )
                tail.append(
                    lambda: bfo_unit(bfo[:, hf, 1, :], outps_f[0:80, 512:1024])
                )
                return tail

            def emit_store(h, hf, bfo, blk=None):
                # stores ride the (nearly idle) sync ring so the scalar
                # queue stays dedicated to evictions
                if blk is None:
                    nc.sync.dma_start(out=o_d[h, hf], in_=bfo[0:65, hf, :, :])
                else:
                    nc.sync.dma_start(
                        out=o_d[h, hf][:, 512 * blk : 512 * blk + 512],
                        in_=bfo[0:65, hf, blk, :],
                    )

            # ---- schedule: loads a full head ahead; stores deferred so no
            # compute queue ever waits on a DMA. Last head runs halves (1,0)
            # so the final tail is the short half's.
            pending_store = []
            carry = None
            bfos = [None] * HPC
            for h in range(HPC):
                if h + 1 < HPC:
                    hd[h + 1] = alloc_head()
                    emit_load_kq(h + 1, hd[h + 1], 0)
                    emit_load_v(h + 1, hd[h + 1])
                order = (0, 1) if h + 1 < HPC else (1, 0)
                bfo = bfos[h] = ep_pool.tile(
                    [80, 2, 2, 512], BF16, tag="bfo", name="bfo"
                )
                carry = emit_half(h, hd[h], order[0], bfo, carry)
                if h + 1 < HPC:
                    emit_load_kq(h + 1, hd[h + 1], 1)
                while pending_store:
                    pending_store.pop(0)()
                carry = emit_half(h, hd[h], order[1], bfo, carry)
                if h + 1 < HPC:
                    pending_store.append(lambda h=h: emit_store(h, 0, bfos[h]))
                    pending_store.append(lambda h=h: emit_store(h, 1, bfos[h]))

            # final flush: carry = last half (hf0) tail [PV(p2), PV(p3),
            # bfo_blk0, bfo_blk1]. Block 0's accumulation finished at kj=3
            # (inside the half), so its eviction + store overlap the block-1
            # PV tail; only block 1's short epilogue remains at the end.
            hl = HPC - 1
            pvs, (bfo_b0, bfo_b1) = carry[:-2], carry[-2:]
            # final half is hf0: block 0 closes at kj=3 (pair 1); pending
            # pairs start at 4 - PV_LAG
            n_b0 = max(0, 1 - (4 - PV_LAG) + 1)
            for pv in pvs[:n_b0]:
                pv()
            bfo_b0()
            emit_store(hl, 1, bfos[hl])
            emit_store(hl, 0, bfos[hl], blk=0)
            for pv in pvs[n_b0:]:
                pv()
            bfo_b1()
            emit_store(hl, 0, bfos[hl], blk=1)

    nc.compile()
    import os

    if os.environ.get("BASS_DEBUG_BALANCE"):
        print(f"balance estimate/core: {T}")
    return nc


_NC = None


def _get_nc():
    global _NC
    if _NC is None:
        _NC = build_attention()
    return _NC


def _to_bf16(x):
    import ml_dtypes

    return np.asarray(x, dtype=np.float32).astype(ml_dtypes.bfloat16)


def _make_in_maps(query, key, value):
    import ml_dtypes

    BH = B * H
    q = _to_bf16(query).reshape(BH, 2, 8, 128, D)
    k = _to_bf16(key).reshape(BH, 2, 8, 128, D)
    # T[h, ph, t, pos] is the SBUF transpose image: t = 0:512 K part
    # (64*blk + dd), t = 512:1536 Q part (128*tile + 64*dup + dd)
    tk = k.transpose(0, 1, 2, 4, 3).reshape(BH, 2, 512, 128)
    qt = q.transpose(0, 1, 2, 4, 3)  # [BH, 2, 8, 64, 128]
    tq = np.broadcast_to(
        qt[:, :, :, None, :, :], (BH, 2, 8, 2, 64, 128)
    ).reshape(BH, 2, 1024, 128)
    t = np.concatenate([tk, tq], axis=2)  # [BH, 2, 1536, 128]
    # SBUF partition image: kqt[h, ph, p, 128*b + c] = T[h, ph, 128*b + p, c]
    kqt = np.ascontiguousarray(
        t.reshape(BH, 2, 12, 128, 128)
        .transpose(0, 1, 3, 2, 4)
        .reshape(BH, 2, 128, 1536)
    )
    # V padded to the stationary-128 layout: [p, blk, 0:64]=V, [.., 64]=1
    v16 = _to_bf16(value).reshape(BH, NT, 128, D)
    vh = np.zeros((BH, 128, NT, 128), dtype=ml_dtypes.bfloat16)
    vh[:, :, :, 0:D] = v16.transpose(0, 2, 1, 3)
    vh[:, :, :, D] = 1.0
    vh = np.ascontiguousarray(vh.reshape(BH, 128, NT * 128))
    return [
        {
            "kqt": kqt[i * HPC : (i + 1) * HPC],
            "value": vh[i * HPC : (i + 1) * HPC],
        }
        for i in range(N_CORES)
    ]


def _post(out_raw):
    # out_raw: [B*H, 2, 65, 1024] bf16: rows 0:64 = out^T, row 64 = denom
    o = np.asarray(out_raw).astype(np.float32)
    num = o[:, :, :D, :]  # [BH, 2, 64, 1024]
    den = o[:, :, D : D + 1, :]  # [BH, 2, 1, 1024]
    res = (num / den).transpose(0, 1, 3, 2)  # [BH, 2, 1024, 64]
    return res.reshape(B, H, S, D).astype(np.float32)


def kernel(query, key, value):
    nc = _get_nc()
    in_maps = _make_in_maps(query, key, value)
    res = run_bass_kernel_spmd(nc, in_maps, core_ids=list(range(N_CORES)))
    out = np.concatenate([res.results[i]["out"] for i in range(N_CORES)], axis=0)
    return _post(out)


# revision 27
# speedup vs baseline: 1.0058x; 1.0058x over previous
"""Causal multi-head attention (B=2, H=16, S=2048, D=64, fp32 I/O) on 8 TRN2
NeuronCores.

Sharding: batch*heads (32 units) split 4-per-core - embarrassingly parallel,
no collectives.

v2 of the per-core kernel (bf16 compute, fp32 PSUM accumulation); measured
~84us typical (vs 109-111us for v1; watch out: run-to-run environment/DVFS
throttling noise of up to +18us was observed on identical binaries - the
`throttle_*` / `ham` fields of the ntff json tell the two apart).
Structural changes vs v1:
  - HOST pre-TRANSPOSES the K|Q layout entirely: the DRAM tensor kqt holds,
    per (head, q-half), exactly the SBUF image the compute wants
    ([128 part, 4 K-pair-slabs x 128 | 8 Q-dup-tiles x 128]), so inputs
    arrive via PLAIN contiguous DMAs (3KB/partition lines) instead of the
    v1 chain of serialized Sync-ring DMA_TRANSPOSEs (which cost a 7.5us PE
    stall at t=15-22us plus a HAM half-duty window to 28.7us).
  - ALL input loads ride ONE hardware DMA queue (the sync ring) in exact
    consumption order (K0 | Q0-tiles-0:4 | Q0-tiles-4:8 | V0 | kq-ph1 |
    head1... ), so the first-needed bytes never lose HBM bandwidth to
    prefetch round-robin across queues. Output stores ride the same ring;
    the scalar queue stays dedicated to evictions.
  - HOST pre-PADS V to 128 stationary columns (V | ones | zeros): V loads
    become fully contiguous 4KB-line DMAs, the on-chip ones-column memset
    disappears, and out^T junk rows become zeros.
  - The per-head epilogue (out^T transpose + reciprocal + scale) moved to
    the HOST: the kernel stores out^T rows 0:65 (64 out dims + denominator
    row from the ones column) straight from the bfo buffer, one plain DMA
    per q-half. Removes the onat DMA_TRANSPOSE (sync ring), reciprocal and
    scale (DVE) per head, and cuts the tail from ~8.4us to ~3us.
  - CROSS-HALF SOFTWARE PIPELINING: each half's tail (its last PV_LAG PV
    batches + 2 bfo evictions) is returned as closures and interleaved
    into the NEXT half's first pairs, so the PE's half-start window (3
    fresh QK pairs racing 6us of evictions) is filled with carried PV
    work. This removed the ~1.2us PE stall per half boundary AND the HAM
    half-duty windows those stalls triggered (full 8/8 duty for the whole
    compute span now).
  - PV lags THREE pairs (not two): PV sits in the in-order PE queue, so a
    deeper lag keeps it from head-of-line blocking on eviction completion.
  - PE warm-up (HAM duty-cycle hold-open) decoupled from the tri01 mask:
    warm weights come from a gpsimd-memset tile so warm matmuls start at
    ~6.5us and run until the first real QK (~10.2us).
  - Last head processes halves (1,0) and the final half's block-0
    eviction + store overlap the block-1 PV tail.
Kept from v1: transposed scores with dual-row-group QK pairing, ScalarE-
exact-Exp / VectorE-Schraudolph eviction balancing, diag-block masks,
PSUM slot rotation (3 score slots + 1 out slot).
Measured dead ends this session: GpSimd cannot access PSUM (BIR verifier
rejects); matmul output must be fp32 (no bf16-PSUM -> no DVE 2x_1p evict);
merging PV matmuls across eviction chunks (per-pair ptiles) coarsens the
evict->PV dependencies enough to re-trigger HAM dips (92.4us); PV_LAG=4
measured worse (86.9); splitting V0 into (0:2)+(2:8) pieces measured
85.1us mean over 3 runs vs 84.7 for the single (0:8) piece - no gain. Split-k PV is blocked by the 8-PSUM-bank
wall (2 accumulators + 3 fp32 score slots = 10 banks) and by HAM
punishing an underutilized PE with half-duty.
"""

import numpy as np

import concourse.bass as bass
import concourse.mybir as mybir
import concourse.tile as tile
from concourse import bacc
from concourse.bass_utils import run_bass_kernel_spmd
from concourse.masks import make_upper_triangular
from concourse.alu_op_type import AluOpType

B, H, S, D = 2, 16, 2048, 64
N_CORES = 8
HPC = (B * H) // N_CORES  # heads per core
NT = S // 128  # 16 k/q blocks of 128
FP32 = mybir.dt.float32
BF16 = mybir.dt.bfloat16
I16 = mybir.dt.int16

LOG2E = 1.4426950408889634
EXP_A = 128.0 * LOG2E / 8.0  # folds softmax scale 1/sqrt(64) into the affine
EXP_B = 127.0 * 128.0 - 5.5
MASK_NEG = -30720.0

SC_BUFS = 3
OPS_BUFS = 1
WARM_MMS = 16
PV_LAG = 3


def _act_cost(el):
    return (el + 170) * 0.8333


def _dve_cost(el, fast=False):
    # fast: DVE 2x_1p mode (all non-scalar operands 2-byte + packed) -
    # applies to the non-diag Schraudolph path on bf16 PSUM scores
    if fast:
        return (0.5 * el + 140) * 1.0417
    return (el + 140) * 1.0417


def build_attention():
    nc = bacc.Bacc("TRN2", target_bir_lowering=False)
    # host supplies bf16 already in the SBUF-image layout per (head, q-half):
    # [128 part, (4 K-pair-slab banks x 128) | (8 Q-dup-tile banks x 128)]
    # so K^T pair-slabs + Q^T (dup'd on both partition halves) land via plain
    # contiguous DMAs - no on-chip transposes, no casts.
    kqt_d = nc.dram_tensor("kqt", [HPC, 2, 128, 1536], BF16, kind="ExternalInput")
    # V padded to 128 cols on host: 0:64 = V^blk, 64 = ones, 65:128 = zeros
    v_d = nc.dram_tensor("value", [HPC, 128, NT * 128], BF16, kind="ExternalInput")
    # out^T per (head, q-half): rows 0:64 = out^T (unscaled), row 64 = softmax
    # denominator; host does the reciprocal-scale and final transpose.
    o_d = nc.dram_tensor("out", [HPC, 2, 65, 1024], BF16, kind="ExternalOutput")

    T = {"act": 0.0, "dve": 0.0, "gps": 0.0}

    with tile.TileContext(nc) as tc:
        with (
            tc.tile_pool(name="singles", bufs=1) as singles,
            tc.tile_pool(name="slab", bufs=3) as slab_pool,
            tc.tile_pool(name="sp1", bufs=1) as sp1_pool,
            tc.tile_pool(name="vp", bufs=3) as v_pool,
            tc.tile_pool(name="sp2", bufs=1) as sp2_pool,
            tc.tile_pool(name="pt", bufs=12) as pt_pool,
            tc.tile_pool(name="ep", bufs=4) as ep_pool,
            tc.tile_pool(name="sc", bufs=SC_BUFS, space="PSUM") as sc_pool,
            tc.tile_pool(name="ops", bufs=OPS_BUFS, space="PSUM") as ops_pool,
        ):
            # spacers: keep DMA-written pools non-adjacent so conservative
            # range-overlap dep tracking never chains unrelated DMAs
            sp1_pool.tile([128, 64], BF16, tag="sp", name="sp1t")
            sp2_pool.tile([128, 64], BF16, tag="sp", name="sp2t")

            # ---- per-head tiles ----
            def alloc_head():
                return {
                    # transposed: [:, 0:4, :] = kslab pairs, [:, 4:12, :] = qt
                    "trs0": slab_pool.tile([128, 12, 128], BF16, tag="trs0", name="t0"),
                    "trs1": slab_pool.tile([128, 12, 128], BF16, tag="trs1", name="t1"),
                    # V padded to 128 cols: 0:64 = V, 64 = ones, 65:128 zeros
                    "vaug": v_pool.tile([128, NT, 128], BF16, tag="vaug", name="va"),
                }

            # ALL input loads go on the sync ring (one hardware DMA queue)
            # in exact consumption order: queue order = transfer order, so
            # the first-needed bytes never lose bandwidth to prefetch.
            def emit_load_kq(h, hd, ph, split=False):
                trs = hd[f"trs{ph}"]
                src = kqt_d[h, ph]
                if split:
                    # K part first, then Q tiles 0-3, so the first QK pair
                    # can fire asap
                    nc.sync.dma_start(out=trs[:, 0:4, :], in_=src[:, 0:512])
                    nc.sync.dma_start(out=trs[:, 4:8, :], in_=src[:, 512:1024])
                    nc.sync.dma_start(out=trs[:, 8:12, :], in_=src[:, 1024:1536])
                else:
                    nc.sync.dma_start(out=trs, in_=src)

            def emit_load_v(h, hd, blocks=None):
                if blocks is None:
                    nc.sync.dma_start(out=hd["vaug"], in_=v_d[h])
                else:
                    a, b = blocks
                    nc.sync.dma_start(
                        out=hd["vaug"][:, a:b, :],
                        in_=v_d[h][:, 128 * a : 128 * b],
                    )

            # ---- head-0 loads FIRST so the DMA queue issues before any
            # mask/warm work ----
            hd = [None] * HPC
            hd[0] = alloc_head()
            emit_load_kq(0, hd[0], 0, split=True)
            emit_load_v(0, hd[0], blocks=(0, 8))
            emit_load_kq(0, hd[0], 1)
            emit_load_v(0, hd[0], blocks=(8, NT))

            # PE HAM warm-up: vector-memset weights (no gpsimd dependency)
            # keep the clock gate open until the first real QK work
            warm = singles.tile([128, 256], BF16, tag="warm")
            nc.gpsimd.memset(warm, 0.0)
            wslot = sc_pool.tile([128, 2, 512], FP32, tag="slot", name="wslot")
            for _ in range(WARM_MMS):
                nc.tensor.matmul(
                    wslot[:, 0, 0:256], warm[:, 0:128], warm, start=True, stop=True
                )

            # ---- one-time masks ----
            tri01 = singles.tile([128, 128], BF16, tag="tri01")
            make_upper_triangular(nc, tri01, val=1.0, diag=True)
            # VectorE fast-exp bias-with-mask: B where keep, -30k where masked
            mbt = singles.tile([128, 128], FP32, tag="mbt")
            nc.gpsimd.memset(mbt, MASK_NEG)
            nc.gpsimd.affine_select(
                out=mbt,
                in_=mbt,
                compare_op=mybir.AluOpType.is_gt,
                fill=EXP_B,
                base=0,
                pattern=[[-1, 128]],
                channel_multiplier=1,
            )
            mb = singles.tile([128, 2, 512], FP32, tag="mb")
            nc.vector.memset(mb, EXP_B)
            nc.vector.tensor_copy(mb[:, 0, 0:128], mbt)
            nc.vector.tensor_copy(mb[:, 1, 128:256], mbt)

            def kslab_ap(hd, rows, kj):
                j = kj // 2
                return hd[f"trs{j // 4}"][rows : rows + 64, j % 4, :]

            def qt_ap(hd, rows, ca, cb):
                hfq = ca // 1024
                trs_f = hd[f"trs{hfq}"].rearrange("p b c -> p (b c)")
                return trs_f[
                    rows : rows + 64,
                    512 + ca - 1024 * hfq : 512 + cb - 1024 * hfq,
                ]

            # ---- eviction units (static greedy ACT/DVE balance) ----
            def evict_unit(slot, ptile, off, cols, diag):
                el = 2 * cols
                if T["act"] + _act_cost(el) <= T["dve"] + _dve_cost(el):
                    T["act"] += _act_cost(el)
                    nc.scalar.activation(
                        ptile[:, :, off : off + cols],
                        slot[:, :, 0:cols],
                        mybir.ActivationFunctionType.Exp,
                        scale=0.125,
                    )
                    if diag:
                        for lane in range(2):
                            nc.gpsimd.tensor_mul(
                                ptile[:, lane, 128 * lane : 128 * lane + 128],
                                ptile[:, lane, 128 * lane : 128 * lane + 128],
                                tri01,
                            )
                        T["gps"] += 940
                else:
                    T["dve"] += _dve_cost(el)
                    p16 = ptile.bitcast(I16)
                    if diag:
                        nc.vector.scalar_tensor_tensor(
                            out=p16[:, :, off : off + cols],
                            in0=slot[:, :, 0:cols],
                            scalar=EXP_A,
                            in1=mb[:, :, 0:cols],
                            op0=AluOpType.mult,
                            op1=AluOpType.add,
                        )
                    else:
                        nc.vector.tensor_scalar(
                            out=p16[:, :, off : off + cols],
                            in0=slot[:, :, 0:cols],
                            scalar1=EXP_A,
                            scalar2=EXP_B,
                            op0=AluOpType.mult,
                            op1=AluOpType.add,
                        )

            def bfo_unit(dst, src):
                # NOTE: GpSimd cannot access PSUM (BIR verifier) - bfo must
                # stay on ACT/DVE
                el = 512
                if T["act"] + _act_cost(el) + 400 <= T["dve"] + _dve_cost(el):
                    T["act"] += _act_cost(el)
                    nc.scalar.copy(dst, src)
                else:
                    T["dve"] += _dve_cost(el)
                    nc.vector.tensor_copy(dst, src)

            # ---- one (head, half) of compute ----
            # Returns the half's TAIL as a list of closures: the last two
            # pairs' PV batches + the two bfo (out^T) evictions. The caller
            # feeds them back as `carry` into the NEXT half, which interleaves
            # them into its first pairs (pj 0-1, before its own PV exists) -
            # this fills the PE's half-start window and lets the eviction
            # engines drain the new half's first slots without stalling QK.
            def emit_half(h, hd, hf, bfo, carry):
                q0 = 1024 * hf
                q1 = q0 + 1024
                kj_hi = 8 * (hf + 1)
                last_kj = [
                    max(
                        kj
                        for kj in range(kj_hi)
                        if max(q0, 128 * kj) < q0 + 512 * (b + 1)
                    )
                    for b in range(2)
                ]

                outps = ops_pool.tile([128, 2, 512], FP32, tag="outps")
                outps_f = outps.rearrange("p a b -> p (a b)")

                def emit_pv(pair, qas, chunks):
                    for lane, (kj, qa) in enumerate(zip(pair, qas)):
                        for ca, cb, ptile in chunks:
                            lo = max(ca, qa)
                            while lo < cb:
                                hi = min(cb, q0 + 512 * ((lo - q0) // 512 + 1))
                                bk = (lo - q0) // 512
                                nc.tensor.matmul(
                                    outps_f[:, lo - q0 : hi - q0],
                                    hd["vaug"][:, kj, :],
                                    ptile[:, lane, lo - ca : hi - ca],
                                    start=(kj == 0),
                                    stop=(kj == last_kj[bk]),
                                )
                                lo = hi

                pending = []
                for pj in range(kj_hi // 2):
                    pair = (2 * pj, 2 * pj + 1)
                    qas = [max(q0, 128 * kj) for kj in pair]
                    diag0 = 128 * pair[0] >= q0
                    chunks = []
                    for ca in range(qas[0], q1, 512):
                        cb = min(ca + 512, q1)
                        cols = cb - ca
                        slot = sc_pool.tile(
                            [128, 2, 512], FP32, tag="slot", name="slot"
                        )
                        for lane, (kj, qa) in enumerate(zip(pair, qas)):
                            lo = max(ca, qa)
                            if lo >= cb:
                                continue
                            rows = (kj % 2) * 64
                            nc.tensor.matmul(
                                slot[:, lane, lo - ca : cols],
                                kslab_ap(hd, rows, kj),
                                qt_ap(hd, rows, lo, cb),
                                start=True,
                                stop=True,
                            )
                        ptile = pt_pool.tile(
                            [128, 2, 512], BF16, tag="ptile", name="ptile"
                        )
                        evict_unit(slot, ptile, 0, cols, diag0 and ca == qas[0])
                        chunks.append((ca, cb, ptile))
                        # one carried tail unit from the previous half per
                        # fresh chunk (the chunk slots in pj 0-2 cover the
                        # carry units)
                        if pj < 3 and carry:
                            carry.pop(0)()
                    pending.append((pair, qas, chunks))
                    # PV lags PV_LAG pairs so evict(j) finishes while
                    # subsequent QK pairs stream (avoids in-order PE queue
                    # head-of-line blocking on the eviction engines)
                    if pj >= PV_LAG:
                        emit_pv(*pending.pop(0))
                while carry:
                    carry.pop(0)()

                tail = [
                    (lambda a=args: emit_pv(*a)) for args in pending
                ]
                tail.append(
                    lambda: bfo_unit(bfo[:, hf, 0, :], outps_f[0:80, 0:512])
                )
                tail.append(
                    lambda: bfo_unit(bfo[:, hf, 1, :], outps_f[0:80, 512:1024])
                )
                return tail

            def emit_store(h, hf, bfo, blk=None):
                # stores ride the (nearly idle) sync ring so the scalar
                # queue stays dedicated to evictions
                if blk is None:
                    nc.sync.dma_start(out=o_d[h, hf], in_=bfo[0:65, hf, :, :])
                else:
                    nc.sync.dma_start(
                        out=o_d[h, hf][:, 512 * blk : 512 * blk + 512],
                        in_=bfo[0:65, hf, blk, :],
                    )

            # ---- schedule: loads a full head ahead; stores deferred so no
            # compute queue ever waits on a DMA. Last head runs halves (1,0)
            # so the final tail is the short half's.
            pending_store = []
            carry = None
            bfos = [None] * HPC
            for h in range(HPC):
                if h + 1 < HPC:
                    hd[h + 1] = alloc_head()
                    emit_load_kq(h + 1, hd[h + 1], 0)
                    emit_load_v(h + 1, hd[h + 1])
                order = (0, 1) if h + 1 < HPC else (1, 0)
                bfo = bfos[h] = ep_pool.tile(
                    [80, 2, 2, 512], BF16, tag="bfo", name="bfo"
                )
                carry = emit_half(h, hd[h], order[0], bfo, carry)
                if h + 1 < HPC:
                    emit_load_kq(h + 1, hd[h + 1], 1)
                while pending_store:
                    pending_store.pop(0)()
                carry = emit_half(h, hd[h], order[1], bfo, carry)
                if h + 1 < HPC:
                    pending_store.append(lambda h=h: emit_store(h, 0, bfos[h]))
                    pending_store.append(lambda h=h: emit_store(h, 1, bfos[h]))

            # final flush: carry = last half (hf0) tail [PV(p2), PV(p3),
            # bfo_blk0, bfo_blk1]. Block 0's accumulation finished at kj=3
            # (inside the half), so its eviction + store overlap the block-1
            # PV tail; only block 1's short epilogue remains at the end.
            hl = HPC - 1
            pvs, (bfo_b0, bfo_b1) = carry[:-2], carry[-2:]
            # final half is hf0: block 0 closes at kj=3 (pair 1); pending
            # pairs start at 4 - PV_LAG
            n_b0 = max(0, 1 - (4 - PV_LAG) + 1)
            for pv in pvs[:n_b0]:
                pv()
            bfo_b0()
            emit_store(hl, 1, bfos[hl])
            emit_store(hl, 0, bfos[hl], blk=0)
            for pv in pvs[n_b0:]:
                pv()
            bfo_b1()
            emit_store(hl, 0, bfos[hl], blk=1)

    nc.compile()
    import os

    if os.environ.get("BASS_DEBUG_BALANCE"):
        print(f"balance estimate/core: {T}")
    return nc


_NC = None


def _get_nc():
    global _NC
    if _NC is None:
        _NC = build_attention()
    return _NC


def _to_bf16(x):
    import ml_dtypes

    return np.asarray(x, dtype=np.float32).astype(ml_dtypes.bfloat16)


def _make_in_maps(query, key, value):
    import ml_dtypes

    BH = B * H
    q = _to_bf16(query).reshape(BH, 2, 8, 128, D)
    k = _to_bf16(key).reshape(BH, 2, 8, 128, D)
    # T[h, ph, t, pos] is the SBUF transpose image: t = 0:512 K part
    # (64*blk + dd), t = 512:1536 Q part (128*tile + 64*dup + dd)
    tk = k.transpose(0, 1, 2, 4, 3).reshape(BH, 2, 512, 128)
    qt = q.transpose(0, 1, 2, 4, 3)  # [BH, 2, 8, 64, 128]
    tq = np.broadcast_to(
        qt[:, :, :, None, :, :], (BH, 2, 8, 2, 64, 128)
    ).reshape(BH, 2, 1024, 128)
    t = np.concatenate([tk, tq], axis=2)  # [BH, 2, 1536, 128]
    # SBUF partition image: kqt[h, ph, p, 128*b + c] = T[h, ph, 128*b + p, c]
    kqt = np.ascontiguousarray(
        t.reshape(BH, 2, 12, 128, 128)
        .transpose(0, 1, 3, 2, 4)
        .reshape(BH, 2, 128, 1536)
    )
    # V padded to the stationary-128 layout: [p, blk, 0:64]=V, [.., 64]=1
    v16 = _to_bf16(value).reshape(BH, NT, 128, D)
    vh = np.zeros((BH, 128, NT, 128), dtype=ml_dtypes.bfloat16)
    vh[:, :, :, 0:D] = v16.transpose(0, 2, 1, 3)
    vh[:, :, :, D] = 1.0
    vh = np.ascontiguousarray(vh.reshape(BH, 128, NT * 128))
    return [
        {
            "kqt": kqt[i * HPC : (i + 1) * HPC],
            "value": vh[i * HPC : (i + 1) * HPC],
        }
        for i in range(N_CORES)
    ]


def _post(out_raw):
    # out_raw: [B*H, 2, 65, 1024] bf16: rows 0:64 = out^T, row 64 = denom
    o = np.asarray(out_raw).astype(np.float32)
    num = o[:, :, :D, :]  # [BH, 2, 64, 1024]
    den = o[:, :, D : D + 1, :]  # [BH, 2, 1, 1024]
    res = (num / den).transpose(0, 1, 3, 2)  # [BH, 2, 1024, 64]
    return res.reshape(B, H, S, D).astype(np.float32)


def kernel(query, key, value):
    nc = _get_nc()
    in_maps = _make_in_maps(query, key, value)
    res = run_bass_kernel_spmd(nc, in_maps, core_ids=list(range(N_CORES)))
    out = np.concatenate([res.results[i]["out"] for i in range(N_CORES)], axis=0)
    return _post(out)
